# revision 47
# baseline (speedup 1.0000x reference)
"""GAT message-passing kernel for 8 Trainium2 NeuronCores (Bass/Tile).

Strategy (v4, "host-gathered edge features"):
  - Host sorts edges by dst and partitions nodes into 8 contiguous ranges;
    each core owns all edges whose dst falls in its range, so segment-softmax
    and scatter-sum are fully core-local (no collectives).
  - Host ships per-edge gathered features hsT = nft[src].T, eftT, hdT =
    nft[dst].T (f16, feature-major, grouped into 128-edge tiles per 128-node
    dst block) plus fused weights Wcat = [W_path | Wz] where
    Wz = W_path @ A2blk + [W_attn1; 0; 0].  All loads are large contiguous
    DMAs; there are no on-device gathers and no precomputed node tables.
  - Per 128-edge tile: three accumulating PE matmuls (stationary = hs/eft/hd
    tile, moving = Wcat 128-row slice) produce PSUM [epaths | z] (136 wide);
    z -> leaky_relu (DVE) -> u = exp(z - SHIFT) (ACT, f16, softmax is
    shift-invariant; no segment max needed); msg = epaths * u (DVE);
    a one-hot P (built on device by a single is_equal per 3-tile batch
    against a host-shipped dstloc table) scatter-matmuls [msg | u] into the
    dst block accumulator [agg | s].
  - Per dst block (tpb tiles): mn = agg / max(s, eps) (deg-0 nodes come out
    as zero automatically), PE-transpose, + nft residual, relu, store to an
    SBUF-resident f16 output stash written back with one DMA at the end.
"""

import sys
import numpy as np
import ml_dtypes

for _p in ("/opt/trn_rl_repo",):
    if _p not in sys.path:
        sys.path.append(_p)

import concourse.bacc as bacc
import concourse.bass as bass
import concourse.mybir as mybir
from concourse.tile import TileContext
from concourse import bass_utils

F = 128
H = 8
DH = 16
FZ = F + H       # 136
NCORES = 8
EXP_SHIFT = 7.0  # exp(a - shift); softmax-invariant, keeps u in f16 range
CH = 36          # edge tiles per DMA chunk (must be a multiple of the 3-tile
                 # batch so a batch never straddles two chunk buffers)
PREC = "f16"


def build_nc(n_nodes, npc, tpb, prec=PREC):
    nb = npc // 128                  # node blocks per core
    ntiles = nb * tpb                # edge tiles per core
    epad = ntiles * 128              # padded edge count per core
    dt = mybir.dt
    f16 = prec == "f16"
    edt = dt.float16 if f16 else dt.float32
    edt_np_bytes = 2 if f16 else 4
    shift = EXP_SHIFT if f16 else 0.0
    AOP = mybir.AluOpType

    nc = bacc.Bacc("TRN2", target_bir_lowering=False, debug=False,
                   num_devices=NCORES)

    # ---- inputs (per-core shards; same shapes on every core) ----
    hsT = nc.dram_tensor("hsT", (F, epad), edt, kind="ExternalInput")
    eftT = nc.dram_tensor("eftT", (F, epad), edt, kind="ExternalInput")
    Wcat = nc.dram_tensor("Wcat", (3 * F, FZ), dt.float32, kind="ExternalInput")
    nftT_c = nc.dram_tensor("nftT_c", (F, npc), edt, kind="ExternalInput")
    # one-hot scatter matrices hold only 0/1 -> exact in fp8, halves their DMA
    pdt = dt.float8e4 if f16 else dt.float32
    Pcat = nc.dram_tensor("Pcat", (128, epad), pdt, kind="ExternalInput")
    PTcat = nc.dram_tensor("PTcat", (128, epad), pdt, kind="ExternalInput")

    outT = nc.dram_tensor("outT", (F, npc), edt, kind="ExternalOutput")

    with TileContext(nc) as tc:
        with tc.tile_pool(name="const", bufs=1) as cpool, \
             tc.tile_pool(name="work", bufs=3) as pool, \
             tc.tile_pool(name="io", bufs=3) as iop, \
             tc.tile_pool(name="psMain", bufs=2, space="PSUM") as psM, \
             tc.tile_pool(name="psB", bufs=2, space="PSUM") as psB, \
             tc.tile_pool(name="psC", bufs=1, space="PSUM") as psC, \
             tc.tile_pool(name="psR", bufs=1, space="PSUM") as psR:

            # ---------- main loop ----------
            chunks = {}

            def load_chunk(c, slices=1, after_first=None):
                if c * CH >= ntiles:
                    return None
                base = c * CH * 128
                w = min(CH * 128, epad - base)
                srcs = (("hs", hsT, edt), ("ef", eftT, edt),
                        ("pc", Pcat, pdt), ("ptc", PTcat, pdt))
                cht = {name: iop.tile([128, CH * 128], dtt, tag=name,
                                      name=name) for name, _, dtt in srcs}
                sw = (w + slices - 1) // slices
                for s in range(0, w, sw):
                    e = min(s + sw, w)
                    for name, dram, _ in srcs:
                        nc.sync.dma_start(out=cht[name][:, s:e],
                                          in_=dram[:, base + s:base + e])
                    if s == 0 and after_first is not None:
                        after_first()
                return cht

            state = {"psb": None}

            def emit_scatter(pend):
                tb_, k3_, msgu_, cht_ = pend
                for k in range(k3_):
                    tg = tb_ + k
                    bb_, jj_ = divmod(tg, tpb)
                    if jj_ == 0:
                        state["psb"] = psB.tile([128, FZ], dt.float32,
                                                tag="agg", name="psb")
                    psb = state["psb"]
                    tk = (tg % CH) * 128
                    nc.tensor.matmul(psb, lhsT=cht_["pc"][:, tk:tk + 128],
                                     rhs=msgu_[:, k, :],
                                     start=(jj_ == 0), stop=(jj_ == tpb - 1),
                                     skip_group_check=True)
                    if jj_ != tpb - 1:
                        continue
                    # ---------- block bb_ epilogue (node-major) ----------
                    # psY[n, f] = sum_fin nft[fin, n] * (W3 + I)[fin, f]
                    #           = y3[n, f] + nft[n, f]   (W3+I baked on host);
                    # y3 is added once per node since sum(att) == 1.
                    psy = psC.tile([128, 128], dt.float32, tag="y3r")
                    nc.tensor.matmul(psy,
                                     lhsT=nft_s[:, bb_ * 128:(bb_ + 1) * 128],
                                     rhs=wcat_s[:, 2, 0:F],
                                     start=True, stop=True,
                                     skip_group_check=True)
                    ss = pool.tile([128, H], dt.float32, tag="ss")
                    nc.vector.tensor_scalar(out=ss, in0=psb[:, F:FZ],
                                            scalar1=1e-30, scalar2=None,
                                            op0=AOP.max)
                    inv = pool.tile([128, H], dt.float32, tag="inv")
                    nc.vector.reciprocal(inv, ss)
                    mn = pool.tile([128, F], dt.float32, tag="mn")
                    nc.vector.tensor_tensor(
                        out=mn[:, :].rearrange("p (h d) -> p h d", h=H),
                        in0=psb[:, 0:F].rearrange("p (h d) -> p h d", h=H),
                        in1=inv[:, :, None].broadcast_to((128, H, DH)),
                        op=AOP.mult)
                    oc = pool.tile([128, 128], edt, tag="oc")
                    nc.vector.tensor_tensor(out=oc, in0=mn, in1=psy,
                                            op=AOP.add)
                    nc.scalar.activation(
                        out_s[:, bb_ * 128:(bb_ + 1) * 128], oc,
                        mybir.ActivationFunctionType.Relu)

            pending = []
            pm = None
            rblk = None
            nshift = wcat_s = nft_s = out_s = None
            for t in range(ntiles):
                c, tc_ = divmod(t, CH)
                if t == 0:
                    def _consts():
                        nonlocal nshift, wcat_s, nft_s, out_s
                        # constants load after the first chunk slice is queued
                        nshift = cpool.tile([128, 1], dt.float32,
                                            name="nshift")
                        nc.vector.memset(nshift, -shift)
                        wcat_f = cpool.tile([128, 3, FZ], dt.float32,
                                            tag="wf", name="wcat_f")
                        nc.sync.dma_start(
                            out=wcat_f,
                            in_=Wcat[:, :].rearrange("(k p) c -> p k c",
                                                     p=128))
                        wcat_s = cpool.tile([128, 3, FZ], edt, name="wcat_s")
                        nc.vector.tensor_copy(out=wcat_s, in_=wcat_f)
                        nft_s = cpool.tile([128, npc], edt, tag="nfts",
                                           name="nft_s")
                        nc.sync.dma_start(out=nft_s, in_=nftT_c[:, :])
                        out_s = cpool.tile([128, npc], edt, tag="outs",
                                           name="out_s")
                    chunks[0] = load_chunk(0, slices=6, after_first=_consts)
                    chunks[1] = load_chunk(1)
                if tc_ == CH // 2:
                    chunks[c + 2] = load_chunk(c + 2)
                    chunks.pop(c - 1, None)
                cht = chunks[c]
                bb, jj = divmod(t, tpb)
                if jj == 0:
                    # rblk = y3A2 for this dst block: nft_blk.T @ W3A2 (8 wide)
                    psr = psR.tile([128, H], dt.float32, tag="rblk")
                    nc.tensor.matmul(psr,
                                     lhsT=nft_s[:, bb * 128:(bb + 1) * 128],
                                     rhs=wcat_s[:, 2, F:FZ],
                                     start=True, stop=True,
                                     skip_group_check=True)
                    rblk = pool.tile([128, H], edt, tag="rbs")
                    nc.vector.tensor_copy(out=rblk, in_=psr)
                t6 = t % 6
                if t6 == 0:
                    # two PSUM banks, 3 tiles of [epaths | z] in each
                    pm = psM.tile([128, 1024], dt.float32, tag="main")
                off = tc_ * 128
                po = (t6 // 3) * 512 + (t6 % 3) * FZ
                for k, name in enumerate(("hs", "ef")):
                    nc.tensor.matmul(pm[:, po:po + FZ],
                                     lhsT=cht[name][:, off:off + 128],
                                     rhs=wcat_s[:, k, :],
                                     start=(k == 0), stop=(k == 1),
                                     skip_group_check=True)
                nc.tensor.matmul(pm[:, po + F:po + FZ],
                                 lhsT=cht["ptc"][:, off:off + 128],
                                 rhs=rblk, start=False, stop=True,
                                 skip_group_check=True)
                if t6 != 5 and t != ntiles - 1:
                    continue

                # ---- batch epilogue: k6 tiles (<= 6) ----
                # Scatter matmuls run two batches behind, so PE is never
                # head-of-line blocked waiting for that batch's ACT/DVE chain.
                if len(pending) == 2:
                    emit_scatter(pending.pop(0))

                k6 = t6 + 1
                tb = t - t6
                kb = (k6 + 2) // 3            # banks used
                kl = k6 - 3 * (kb - 1)        # tiles in last bank
                # 4D views of the two banks: [p, bank, tile, c]
                pz = pm.rearrange("p (b c) -> p b c", c=512)[:, 0:kb, 0:3 * FZ] \
                       .rearrange("p b (k c) -> p b k c", c=FZ)
                # u = exp(leaky_relu(z) - s) == max(exp(z - s), exp(0.01*z - s))
                # (exp is monotone), both exps read PSUM directly on ScalarE.
                u1 = pool.tile([128, 6, H], edt, tag="u1")
                u1v = u1.rearrange("p (b k) h -> p b k h", b=2)
                u2 = pool.tile([128, 6, H], edt, tag="u2")
                u2v = u2.rearrange("p (b k) h -> p b k h", b=2)
                nc.scalar.activation(
                    u1v[:, 0:kb, :, :], pz[:, :, :, F:FZ],
                    mybir.ActivationFunctionType.Exp, bias=nshift[:, :])
                nc.scalar.activation(
                    u2v[:, 0:kb, :, :], pz[:, :, :, F:FZ],
                    mybir.ActivationFunctionType.Exp, bias=nshift[:, :],
                    scale=0.01)
                msgu6 = pool.tile([128, 6, FZ], edt, tag="msgu6")
                nc.vector.tensor_tensor(
                    out=msgu6[:, 0:3 * kb, F:FZ], in0=u1[:, 0:3 * kb, :],
                    in1=u2[:, 0:3 * kb, :], op=AOP.max)
                # Bank 0: ScalarE downcasts epaths PSUM->f16, DVE multiplies at
                # the 2 elem/cycle f16 rate.  Bank 1: DVE multiplies straight
                # from PSUM f32.  Splitting keeps both engines below TensorE.
                part3 = pool.tile([128, 3, F], edt, tag="part3")
                nc.scalar.activation(part3, pz[:, 0, :, 0:F],
                                     mybir.ActivationFunctionType.Copy)
                nc.vector.tensor_tensor(
                    out=msgu6[:, 0:3, 0:F]
                        .rearrange("p k (h d) -> p k h d", h=H),
                    in0=part3.rearrange("p k (h d) -> p k h d", h=H),
                    in1=msgu6[:, 0:3, F:FZ][:, :, :, None]
                        .broadcast_to((128, 3, H, DH)),
                    op=AOP.mult)
                if kb > 1:
                    nc.vector.tensor_tensor(
                        out=msgu6[:, 3:6, 0:F]
                            .rearrange("p k (h d) -> p k h d", h=H),
                        in0=pz[:, 1, :, 0:F]
                            .rearrange("p k (h d) -> p k h d", h=H),
                        in1=msgu6[:, 3:6, F:FZ][:, :, :, None]
                            .broadcast_to((128, 3, H, DH)),
                        op=AOP.mult)
                pending.append((tb, k6, msgu6, cht))

            for p_ in pending:
                emit_scatter(p_)
            nc.sync.dma_start(out=outT[:, :], in_=out_s)

    nc.compile()
    return nc


def prep_inputs(nft, eft, W_path, b_path, W_attn1, attn2, src, dst,
                npc, tpb, prec=PREC):
    """Host-side sharding/relayout. Returns (in_maps, meta)."""
    n_nodes = nft.shape[0]
    nb = npc // 128
    ntiles = nb * tpb
    epad = ntiles * 128
    edt_np = np.float16 if prec == "f16" else np.float32

    nft = np.ascontiguousarray(nft, dtype=np.float32)
    eft = np.ascontiguousarray(eft, dtype=np.float32)
    src = np.asarray(src, dtype=np.int64)
    dst = np.asarray(dst, dtype=np.int64)
    perm = np.argsort(dst, kind="stable")
    sdst = dst[perm]
    ssrc = src[perm]

    # fused weights [W_path | Wz]; fold bias into nothing (b_path==0 checked)
    a2 = np.asarray(attn2, dtype=np.float32).reshape(H, DH)
    A2blk = np.zeros((F, H), dtype=np.float32)
    for h in range(H):
        A2blk[h * DH:(h + 1) * DH, h] = a2[h]
    Wp = np.asarray(W_path, dtype=np.float32)
    Wz = Wp @ A2blk
    Wz[0:F] += np.asarray(W_attn1, dtype=np.float32)
    Wcat = np.concatenate([Wp, Wz], axis=1).copy()  # [384, 136] f32
    # W3 block gains +I: the per-block matmul then yields y3 + nft (the
    # residual) in one shot.  The z columns (W3@A2blk) stay pure.
    Wcat[2 * F:3 * F, 0:F] += np.eye(F, dtype=np.float32)

    has_bias = bool(np.any(np.asarray(b_path) != 0))
    assert not has_bias, "bias path not implemented in v4 kernel"

    nftT16 = np.ascontiguousarray(nft.T.astype(edt_np))      # [F, N]
    eftT16 = np.ascontiguousarray(eft.T.astype(edt_np))      # [F, E]

    in_maps = []
    meta = []
    for c in range(NCORES):
        lo = c * npc
        hi = min((c + 1) * npc, n_nodes)

        eidx = np.full(epad, -1, dtype=np.int64)   # sorted-edge id per slot
        dstloc = np.full(epad, 999, dtype=np.int64)
        for b_i in range(nb):
            base = lo + b_i * 128
            if base >= n_nodes:
                continue
            s = np.searchsorted(sdst, base)
            e = np.searchsorted(sdst, min(base + 128, n_nodes))
            cnt = e - s
            assert cnt <= tpb * 128, f"block overflow: {cnt} > {tpb * 128}"
            o = b_i * tpb * 128
            eidx[o:o + cnt] = np.arange(s, e)
            dstloc[o:o + cnt] = sdst[s:e] - base

        valid = eidx >= 0
        e_sorted = np.where(valid, eidx, 0)
        src_cols = np.where(valid, ssrc[e_sorted], 0)
        dst_cols = np.where(valid, sdst[e_sorted], 0)
        edge_cols = np.where(valid, perm[e_sorted], 0)

        ee = np.arange(epad)
        vv = ee[valid]
        pdt_np = ml_dtypes.float8_e4m3 if prec == "f16" else np.float32
        Pc = np.zeros((128, epad), dtype=pdt_np)
        Pc[vv % 128, (vv // 128) * 128 + dstloc[vv]] = 1.0
        PTc = np.zeros((128, epad), dtype=pdt_np)
        PTc[dstloc[vv], vv] = 1.0

        m = {
            "hsT": np.ascontiguousarray(nftT16[:, src_cols]),
            "eftT": np.ascontiguousarray(eftT16[:, edge_cols]),
            "Wcat": Wcat,
            "nftT_c": np.zeros((F, npc), dtype=edt_np),
            "Pcat": Pc,
            "PTcat": PTc,
        }
        m["nftT_c"][:, :hi - lo] = nftT16[:, lo:hi]
        in_maps.append(m)
        meta.append((lo, hi))
    return in_maps, meta


_NC_CACHE = {}


def _get_nc(key, *args, **kw):
    if key not in _NC_CACHE:
        _NC_CACHE[key] = build_nc(*args, **kw)
    return _NC_CACHE[key]


def run(nft, eft, W_path, b_path, W_attn1, attn2, src, dst, trace=False,
        tmpdir=None, prec=PREC):
    n_nodes = nft.shape[0]
    npc = ((n_nodes + NCORES - 1) // NCORES + 127) // 128 * 128
    dst64 = np.asarray(dst, dtype=np.int64)
    cnt = np.bincount(dst64, minlength=((n_nodes + 127) // 128) * 128)
    blocks = cnt.reshape(-1, 128).sum(axis=1)
    tpb = int(np.ceil(blocks.max() / 128.0)) if blocks.max() > 0 else 1

    in_maps, meta = prep_inputs(
        np.asarray(nft), np.asarray(eft), np.asarray(W_path),
        np.asarray(b_path), np.asarray(W_attn1), np.asarray(attn2),
        np.asarray(src), dst64, npc, tpb, prec=prec)

    nc = _get_nc((n_nodes, npc, tpb, prec), n_nodes, npc, tpb, prec=prec)
    kw = {}
    if trace:
        kw = dict(trace=True, tmpdir=tmpdir)
    res = bass_utils.run_bass_kernel_spmd(nc, in_maps,
                                          core_ids=list(range(NCORES)), **kw)

    nb = npc // 128
    out = np.empty((n_nodes, F), dtype=np.float32)
    for c, (lo, hi) in enumerate(meta):
        # outT is node-major per block: outT[p, b*128 + f] = out[b*128+p, f]
        o = res.results[c]["outT"].reshape(128, nb, F).transpose(1, 0, 2)
        out[lo:hi] = o.reshape(npc, F)[:hi - lo].astype(np.float32)
    # deg-0 nodes: kernel adds y3 unconditionally (sum att == 1 assumption);
    # fix the (rare) isolated nodes exactly: out = relu(nft).
    deg = np.bincount(dst64, minlength=n_nodes)
    iso = deg == 0
    if iso.any():
        out[iso] = np.maximum(np.asarray(nft, dtype=np.float32)[iso], 0.0)
    return out, res


def kernel(**inputs):
    out, _ = run(**inputs)
    return out


# revision 50
# speedup vs baseline: 1.0956x; 1.0956x over previous
"""GAT message-passing kernel for 8 Trainium2 NeuronCores (Bass/Tile).

Strategy (v4, "host-gathered edge features"):
  - Host sorts edges by dst and partitions nodes into 8 contiguous ranges;
    each core owns all edges whose dst falls in its range, so segment-softmax
    and scatter-sum are fully core-local (no collectives).
  - Host ships per-edge gathered features hsT = nft[src].T, eftT, hdT =
    nft[dst].T (f16, feature-major, grouped into 128-edge tiles per 128-node
    dst block) plus fused weights Wcat = [W_path | Wz] where
    Wz = W_path @ A2blk + [W_attn1; 0; 0].  All loads are large contiguous
    DMAs; there are no on-device gathers and no precomputed node tables.
  - Per 128-edge tile: three accumulating PE matmuls (stationary = hs/eft/hd
    tile, moving = Wcat 128-row slice) produce PSUM [epaths | z] (136 wide);
    z -> leaky_relu (DVE) -> u = exp(z - SHIFT) (ACT, f16, softmax is
    shift-invariant; no segment max needed); msg = epaths * u (DVE);
    a one-hot P (built on device by a single is_equal per 3-tile batch
    against a host-shipped dstloc table) scatter-matmuls [msg | u] into the
    dst block accumulator [agg | s].
  - Per dst block (tpb tiles): mn = agg / max(s, eps) (deg-0 nodes come out
    as zero automatically), PE-transpose, + nft residual, relu, store to an
    SBUF-resident f16 output stash written back with one DMA at the end.
"""

import sys
import numpy as np
import ml_dtypes

for _p in ("/opt/trn_rl_repo",):
    if _p not in sys.path:
        sys.path.append(_p)

import concourse.bacc as bacc
import concourse.bass as bass
import concourse.mybir as mybir
from concourse.tile import TileContext
from concourse import bass_utils

F = 128
H = 8
DH = 16
FZ = F + H       # 136 (scatter rhs width: [msg | u])
FZP = F + 2 * H  # 144 (psum tile width: [epaths | z | 0.01*z])
NCORES = 8
EXP_SHIFT = 7.0  # exp(a - shift); softmax-invariant, keeps u in f16 range
CH = 36          # edge tiles per DMA chunk (must be a multiple of the 3-tile
                 # batch so a batch never straddles two chunk buffers)
PREC = "f16"


def build_nc(n_nodes, npc, tpb, prec=PREC):
    nb = npc // 128                  # node blocks per core
    ntiles = nb * tpb                # edge tiles per core
    epad = ntiles * 128              # padded edge count per core
    dt = mybir.dt
    f16 = prec == "f16"
    edt = dt.float16 if f16 else dt.float32
    edt_np_bytes = 2 if f16 else 4
    shift = EXP_SHIFT if f16 else 0.0
    AOP = mybir.AluOpType

    nc = bacc.Bacc("TRN2", target_bir_lowering=False, debug=False,
                   num_devices=NCORES)

    # ---- inputs (per-core shards; same shapes on every core) ----
    hsT = nc.dram_tensor("hsT", (F, epad), edt, kind="ExternalInput")
    eftT = nc.dram_tensor("eftT", (F, epad), edt, kind="ExternalInput")
    Wcat = nc.dram_tensor("Wcat", (3 * F, FZP), dt.float32,
                          kind="ExternalInput")
    nftT_c = nc.dram_tensor("nftT_c", (F, npc), edt, kind="ExternalInput")
    # one-hot scatter matrices hold only 0/1 -> exact in fp8, halves their DMA
    pdt = dt.float8e4 if f16 else dt.float32
    Pcat = nc.dram_tensor("Pcat", (128, epad), pdt, kind="ExternalInput")
    PTcat = nc.dram_tensor("PTcat", (128, epad), pdt, kind="ExternalInput")

    outT = nc.dram_tensor("outT", (F, npc), edt, kind="ExternalOutput")

    with TileContext(nc) as tc:
        with tc.tile_pool(name="const", bufs=1) as cpool, \
             tc.tile_pool(name="work", bufs=3) as pool, \
             tc.tile_pool(name="io", bufs=3) as iop, \
             tc.tile_pool(name="psMain", bufs=2, space="PSUM") as psM, \
             tc.tile_pool(name="psB", bufs=2, space="PSUM") as psB, \
             tc.tile_pool(name="psC", bufs=1, space="PSUM") as psC, \
             tc.tile_pool(name="psR", bufs=1, space="PSUM") as psR:

            # ---------- main loop ----------
            chunks = {}

            def load_chunk(c, slices=1, after_first=None):
                if c * CH >= ntiles:
                    return None
                base = c * CH * 128
                w = min(CH * 128, epad - base)
                srcs = (("hs", hsT, edt), ("ef", eftT, edt),
                        ("pc", Pcat, pdt), ("ptc", PTcat, pdt))
                cht = {name: iop.tile([128, CH * 128], dtt, tag=name,
                                      name=name) for name, _, dtt in srcs}
                sw = (w + slices - 1) // slices
                for s in range(0, w, sw):
                    e = min(s + sw, w)
                    for name, dram, _ in srcs:
                        nc.sync.dma_start(out=cht[name][:, s:e],
                                          in_=dram[:, base + s:base + e])
                    if s == 0 and after_first is not None:
                        after_first()
                return cht

            state = {"psb": None}

            def emit_scatter(pend):
                tb_, k3_, msgu_, cht_ = pend
                for k in range(k3_):
                    tg = tb_ + k
                    bb_, jj_ = divmod(tg, tpb)
                    if jj_ == 0:
                        state["psb"] = psB.tile([128, FZ], dt.float32,
                                                tag="agg", name="psb")
                    psb = state["psb"]
                    tk = (tg % CH) * 128
                    nc.tensor.matmul(psb, lhsT=cht_["pc"][:, tk:tk + 128],
                                     rhs=msgu_[:, k, :],
                                     start=(jj_ == 0), stop=(jj_ == tpb - 1),
                                     skip_group_check=True)
                    if jj_ != tpb - 1:
                        continue
                    # ---------- block bb_ epilogue (node-major) ----------
                    # psY[n, f] = sum_fin nft[fin, n] * (W3 + I)[fin, f]
                    #           = y3[n, f] + nft[n, f]   (W3+I baked on host);
                    # y3 is added once per node since sum(att) == 1.
                    psy = psC.tile([128, 128], dt.float32, tag="y3r")
                    nc.tensor.matmul(psy,
                                     lhsT=nft_s[:, bb_ * 128:(bb_ + 1) * 128],
                                     rhs=wcat_s[:, 2, 0:F],
                                     start=True, stop=True,
                                     skip_group_check=True)
                    ss = pool.tile([128, H], dt.float32, tag="ss")
                    nc.vector.tensor_scalar(out=ss, in0=psb[:, F:FZ],
                                            scalar1=1e-30, scalar2=None,
                                            op0=AOP.max)
                    inv = pool.tile([128, H], dt.float32, tag="inv")
                    nc.vector.reciprocal(inv, ss)
                    mn = pool.tile([128, F], dt.float32, tag="mn")
                    nc.vector.tensor_tensor(
                        out=mn[:, :].rearrange("p (h d) -> p h d", h=H),
                        in0=psb[:, 0:F].rearrange("p (h d) -> p h d", h=H),
                        in1=inv[:, :, None].broadcast_to((128, H, DH)),
                        op=AOP.mult)
                    oc = pool.tile([128, 128], edt, tag="oc")
                    nc.vector.tensor_tensor(out=oc, in0=mn, in1=psy,
                                            op=AOP.add)
                    nc.scalar.activation(
                        out_s[:, bb_ * 128:(bb_ + 1) * 128], oc,
                        mybir.ActivationFunctionType.Relu)

            pending = []
            pm = None
            rblk = None
            nshift = wcat_s = nft_s = out_s = None
            for t in range(ntiles):
                c, tc_ = divmod(t, CH)
                if t == 0:
                    def _consts():
                        nonlocal nshift, wcat_s, nft_s, out_s
                        # constants load after the first chunk slice is queued
                        nshift = cpool.tile([128, 1], dt.float32,
                                            name="nshift")
                        nc.vector.memset(nshift, -shift)
                        wcat_f = cpool.tile([128, 3, FZP], dt.float32,
                                            tag="wf", name="wcat_f")
                        nc.sync.dma_start(
                            out=wcat_f,
                            in_=Wcat[:, :].rearrange("(k p) c -> p k c",
                                                     p=128))
                        wcat_s = cpool.tile([128, 3, FZP], edt,
                                            name="wcat_s")
                        nc.vector.tensor_copy(out=wcat_s, in_=wcat_f)
                        nft_s = cpool.tile([128, npc], edt, tag="nfts",
                                           name="nft_s")
                        nc.sync.dma_start(out=nft_s, in_=nftT_c[:, :])
                        out_s = cpool.tile([128, npc], edt, tag="outs",
                                           name="out_s")
                    chunks[0] = load_chunk(0, slices=6, after_first=_consts)
                    chunks[1] = load_chunk(1)
                if tc_ == CH // 2:
                    chunks[c + 2] = load_chunk(c + 2)
                    chunks.pop(c - 1, None)
                cht = chunks[c]
                bb, jj = divmod(t, tpb)
                if jj == 0:
                    # rblk = y3A2 for this dst block: nft_blk.T @ W3A2 (8 wide)
                    psr = psR.tile([128, 2 * H], dt.float32, tag="rblk")
                    nc.tensor.matmul(psr,
                                     lhsT=nft_s[:, bb * 128:(bb + 1) * 128],
                                     rhs=wcat_s[:, 2, F:FZP],
                                     start=True, stop=True,
                                     skip_group_check=True)
                    rblk = pool.tile([128, 2 * H], edt, tag="rbs")
                    nc.vector.tensor_copy(out=rblk, in_=psr)
                t6 = t % 6
                if t6 == 0:
                    # two PSUM banks, 3 tiles of [epaths | z] in each
                    pm = psM.tile([128, 1024], dt.float32, tag="main")
                off = tc_ * 128
                po = (t6 // 3) * 512 + (t6 % 3) * FZP
                for k, name in enumerate(("hs", "ef")):
                    nc.tensor.matmul(pm[:, po:po + FZP],
                                     lhsT=cht[name][:, off:off + 128],
                                     rhs=wcat_s[:, k, :],
                                     start=(k == 0), stop=(k == 1),
                                     skip_group_check=True)
                nc.tensor.matmul(pm[:, po + F:po + FZP],
                                 lhsT=cht["ptc"][:, off:off + 128],
                                 rhs=rblk, start=False, stop=True,
                                 skip_group_check=True)
                if t6 != 5 and t != ntiles - 1:
                    continue

                # ---- batch epilogue: k6 tiles (<= 6) ----
                # Scatter matmuls run two batches behind, so PE is never
                # head-of-line blocked waiting for that batch's ACT/DVE chain.
                if len(pending) == 2:
                    emit_scatter(pending.pop(0))

                k6 = t6 + 1
                tb = t - t6
                kb = (k6 + 2) // 3            # banks used
                kl = k6 - 3 * (kb - 1)        # tiles in last bank
                # 4D views of the two banks: [p, bank, tile, c]
                pz = pm.rearrange("p (b c) -> p b c", c=512)[:, 0:kb, 0:3 * FZP] \
                       .rearrange("p b (k c) -> p b k c", c=FZP)
                # u = exp(leaky_relu(z) - s) == max(exp(z - s), exp(0.01*z - s))
                # (exp is monotone); the weights emit both z and 0.01*z, so a
                # single Exp covers both operands of the max.
                u12 = pool.tile([128, 6, 2 * H], edt, tag="u12")
                u12v = u12.rearrange("p (b k) h -> p b k h", b=2)
                nc.scalar.activation(
                    u12v[:, 0:kb, :, :], pz[:, :, :, F:FZP],
                    mybir.ActivationFunctionType.Exp, bias=nshift[:, :])
                msgu6 = pool.tile([128, 6, FZ], edt, tag="msgu6")
                nc.vector.tensor_tensor(
                    out=msgu6[:, 0:3 * kb, F:FZ],
                    in0=u12[:, 0:3 * kb, 0:H],
                    in1=u12[:, 0:3 * kb, H:2 * H], op=AOP.max)
                # ScalarE downcasts epaths PSUM->f16 so the DVE multiply runs
                # at the 2 elem/cycle f16 rate instead of 1/cycle PSUM-f32.
                part6 = pool.tile([128, 6, F], edt, tag="part6")
                p6v = part6.rearrange("p (b k) c -> p b k c", b=2)
                nc.scalar.activation(p6v[:, 0:kb, :, :], pz[:, :, :, 0:F],
                                     mybir.ActivationFunctionType.Copy)
                nc.vector.tensor_tensor(
                    out=msgu6[:, 0:3 * kb, 0:F]
                        .rearrange("p k (h d) -> p k h d", h=H),
                    in0=part6[:, 0:3 * kb, :]
                        .rearrange("p k (h d) -> p k h d", h=H),
                    in1=msgu6[:, 0:3 * kb, F:FZ][:, :, :, None]
                        .broadcast_to((128, 3 * kb, H, DH)),
                    op=AOP.mult)
                pending.append((tb, k6, msgu6, cht))

            for p_ in pending:
                emit_scatter(p_)
            nc.sync.dma_start(out=outT[:, :], in_=out_s)

    nc.compile()
    return nc


def prep_inputs(nft, eft, W_path, b_path, W_attn1, attn2, src, dst,
                npc, tpb, prec=PREC):
    """Host-side sharding/relayout. Returns (in_maps, meta)."""
    n_nodes = nft.shape[0]
    nb = npc // 128
    ntiles = nb * tpb
    epad = ntiles * 128
    edt_np = np.float16 if prec == "f16" else np.float32

    nft = np.ascontiguousarray(nft, dtype=np.float32)
    eft = np.ascontiguousarray(eft, dtype=np.float32)
    src = np.asarray(src, dtype=np.int64)
    dst = np.asarray(dst, dtype=np.int64)
    perm = np.argsort(dst, kind="stable")
    sdst = dst[perm]
    ssrc = src[perm]

    # fused weights [W_path | Wz]; fold bias into nothing (b_path==0 checked)
    a2 = np.asarray(attn2, dtype=np.float32).reshape(H, DH)
    A2blk = np.zeros((F, H), dtype=np.float32)
    for h in range(H):
        A2blk[h * DH:(h + 1) * DH, h] = a2[h]
    Wp = np.asarray(W_path, dtype=np.float32)
    Wz = Wp @ A2blk
    Wz[0:F] += np.asarray(W_attn1, dtype=np.float32)
    # [W_path | Wz | 0.01*Wz]: the scaled copy lets one Exp produce both
    # operands of max(exp(z - s), exp(0.01z - s)) == exp(leaky_relu(z) - s).
    Wcat = np.concatenate([Wp, Wz, 0.01 * Wz], axis=1).copy()  # [384, 144]
    # W3 block gains +I: the per-block matmul then yields y3 + nft (the
    # residual) in one shot.  The z columns (W3@A2blk) stay pure.
    Wcat[2 * F:3 * F, 0:F] += np.eye(F, dtype=np.float32)

    has_bias = bool(np.any(np.asarray(b_path) != 0))
    assert not has_bias, "bias path not implemented in v4 kernel"

    nftT16 = np.ascontiguousarray(nft.T.astype(edt_np))      # [F, N]
    eftT16 = np.ascontiguousarray(eft.T.astype(edt_np))      # [F, E]

    in_maps = []
    meta = []
    for c in range(NCORES):
        lo = c * npc
        hi = min((c + 1) * npc, n_nodes)

        eidx = np.full(epad, -1, dtype=np.int64)   # sorted-edge id per slot
        dstloc = np.full(epad, 999, dtype=np.int64)
        for b_i in range(nb):
            base = lo + b_i * 128
            if base >= n_nodes:
                continue
            s = np.searchsorted(sdst, base)
            e = np.searchsorted(sdst, min(base + 128, n_nodes))
            cnt = e - s
            assert cnt <= tpb * 128, f"block overflow: {cnt} > {tpb * 128}"
            o = b_i * tpb * 128
            eidx[o:o + cnt] = np.arange(s, e)
            dstloc[o:o + cnt] = sdst[s:e] - base

        valid = eidx >= 0
        e_sorted = np.where(valid, eidx, 0)
        src_cols = np.where(valid, ssrc[e_sorted], 0)
        dst_cols = np.where(valid, sdst[e_sorted], 0)
        edge_cols = np.where(valid, perm[e_sorted], 0)

        ee = np.arange(epad)
        vv = ee[valid]
        pdt_np = ml_dtypes.float8_e4m3 if prec == "f16" else np.float32
        Pc = np.zeros((128, epad), dtype=pdt_np)
        Pc[vv % 128, (vv // 128) * 128 + dstloc[vv]] = 1.0
        PTc = np.zeros((128, epad), dtype=pdt_np)
        PTc[dstloc[vv], vv] = 1.0

        m = {
            "hsT": np.ascontiguousarray(nftT16[:, src_cols]),
            "eftT": np.ascontiguousarray(eftT16[:, edge_cols]),
            "Wcat": Wcat,
            "nftT_c": np.zeros((F, npc), dtype=edt_np),
            "Pcat": Pc,
            "PTcat": PTc,
        }
        m["nftT_c"][:, :hi - lo] = nftT16[:, lo:hi]
        in_maps.append(m)
        meta.append((lo, hi))
    return in_maps, meta


_NC_CACHE = {}


def _get_nc(key, *args, **kw):
    if key not in _NC_CACHE:
        _NC_CACHE[key] = build_nc(*args, **kw)
    return _NC_CACHE[key]


def run(nft, eft, W_path, b_path, W_attn1, attn2, src, dst, trace=False,
        tmpdir=None, prec=PREC):
    n_nodes = nft.shape[0]
    npc = ((n_nodes + NCORES - 1) // NCORES + 127) // 128 * 128
    dst64 = np.asarray(dst, dtype=np.int64)
    cnt = np.bincount(dst64, minlength=((n_nodes + 127) // 128) * 128)
    blocks = cnt.reshape(-1, 128).sum(axis=1)
    tpb = int(np.ceil(blocks.max() / 128.0)) if blocks.max() > 0 else 1

    in_maps, meta = prep_inputs(
        np.asarray(nft), np.asarray(eft), np.asarray(W_path),
        np.asarray(b_path), np.asarray(W_attn1), np.asarray(attn2),
        np.asarray(src), dst64, npc, tpb, prec=prec)

    nc = _get_nc((n_nodes, npc, tpb, prec), n_nodes, npc, tpb, prec=prec)
    kw = {}
    if trace:
        kw = dict(trace=True, tmpdir=tmpdir)
    res = bass_utils.run_bass_kernel_spmd(nc, in_maps,
                                          core_ids=list(range(NCORES)), **kw)

    nb = npc // 128
    out = np.empty((n_nodes, F), dtype=np.float32)
    for c, (lo, hi) in enumerate(meta):
        # outT is node-major per block: outT[p, b*128 + f] = out[b*128+p, f]
        o = res.results[c]["outT"].reshape(128, nb, F).transpose(1, 0, 2)
        out[lo:hi] = o.reshape(npc, F)[:hi - lo].astype(np.float32)
    # deg-0 nodes: kernel adds y3 unconditionally (sum att == 1 assumption);
    # fix the (rare) isolated nodes exactly: out = relu(nft).
    deg = np.bincount(dst64, minlength=n_nodes)
    iso = deg == 0
    if iso.any():
        out[iso] = np.maximum(np.asarray(nft, dtype=np.float32)[iso], 0.0)
    return out, res


def kernel(**inputs):
    out, _ = run(**inputs)
    return out
